# revision 13
# baseline (speedup 1.0000x reference)
"""Trainium2 Bass kernel for DeepMultiOmicPathwayNet.

Model (per batch row n):
  g    = x[n, pathway_ids, :]                  -> [P=200, K*C=192]
  t    = einsum('pi,pio->po', g, W_path) + b_path      (per-pathway linear)
  t    = t / ||t||_2 (row L2 over each pathway's 64 outputs)
  ncb  = x[n, nc_ids, :].flatten() @ W_nc + b_nc       ([15000] @ [15000,512])
  h    = sigmoid(concat(t.flatten(), ncb))             ([13312])
  out  = h @ W_out + b_out                             ([20])

Strategy: data-parallel over batch N=1024 across 8 cores (128 rows/core).
DMA-bound problem; v6 design:
  - All large tensors ship as fp8e4m3 (~2x fewer HBM bytes than bf16).
    W_path/b_path pre-scaled x8 (exactly cancelled by the L2 norm);
    W_nc/b_nc pre-scaled x64 (compensated via sigmoid's scale=1/64).
  - nc gather dedup: repeated gene ids are shipped once, their W_nc rows
    folded by summation on the host (exact).
  - nc-branch matmuls use fp8 DoubleRow perf mode (0.5 cycles/row).
  - Data and weights are packed into shared DRAM tensors so each chunk is
    ONE large DMA (~23 DMAs total); pathway data first in the stream (it
    feeds the long DVE/ACT elementwise pipeline), nc stream last.
  - Elementwise work batched over PAIRS of 8-pathway groups ([128,1024]
    per op) to halve instruction count and amortize access latencies.
  - rsqrt via Quake bit-trick + 1 Newton step on DVE (max err ~0.17%),
    batched per 4-group sub-phase: no ACT table swaps, no phase split.
  - Software pipelining: the transpose/copy/W_out stage of sub-phase k-1
    is emitted between sub-phase k's reduce and normalize stages.
"""
import numpy as np
import ml_dtypes

import concourse.bass as bass
import concourse.bacc as bacc
import concourse.tile as tile
import concourse.mybir as mybir
from concourse.bass_utils import run_bass_kernel_spmd
from concourse.masks import make_identity

bf16 = mybir.dt.bfloat16
f32 = mybir.dt.float32
i32 = mybir.dt.int32
f8 = mybir.dt.float8e4
BF = ml_dtypes.bfloat16
F8 = ml_dtypes.float8_e4m3
AF = mybir.ActivationFunctionType
ALU = mybir.AluOpType
DR = mybir.MatmulPerfMode.DoubleRow

N, G, C = 1024, 20000, 3
P, K = 200, 64
KC = K * C              # 192
NCG = 5000              # non-cancer genes
HID = 512
OUT = 20
NB = 128                # batch rows per core
NCORES = 8
NGRP = P // 8           # 25 groups of 8 pathways
NFT = P * K // 128      # 100 feature tiles from pathways
NFT_NC = HID // 128     # 4 feature tiles from nc branch
MAGIC = 0x5F3759DF      # fast inverse sqrt

_CACHE = {}


def _build(NKT):
    NKP = NKT // 2      # DoubleRow k-tile pairs
    nc = bacc.Bacc(None, target_bir_lowering=False)

    # data + weights packed per pathway: [..., 0:128]=x rows, [128:192]=W
    pwh_d = nc.declare_dram_parameter("pwh", [128, NGRP, 8, 192], f8, isOutput=False)
    pwl_d = nc.declare_dram_parameter("pwl", [65, NGRP, 8, 192], f8, isOutput=False)
    # nc branch packed per k-tile: [..., 0:128]=x rows, [128:640]=W_nc
    ncw_d = nc.declare_dram_parameter("ncw", [128, NKT, 640], f8, isOutput=False)
    wout_d = nc.declare_dram_parameter("wout", [128, NFT + NFT_NC, OUT], bf16, isOutput=False)
    bout_d = nc.declare_dram_parameter("bout", [1, OUT], bf16, isOutput=False)
    out_d = nc.declare_dram_parameter("out", [NB, OUT], f32, isOutput=True)

    with tile.TileContext(nc) as tc:
        with (
            tc.tile_pool(name="cst", bufs=1) as cst,
            tc.tile_pool(name="sqp", bufs=2) as sqp,
            tc.tile_pool(name="tnp", bufs=2) as tnp,
            tc.tile_pool(name="s8p", bufs=4) as s8p,
            tc.tile_pool(name="htp", bufs=2) as htp,
            tc.tile_pool(name="ivp", bufs=2) as ivp,
            tc.tile_pool(name="tp", bufs=5, space="PSUM") as tp,
            tc.tile_pool(name="stp", bufs=1, space="PSUM") as stp,
            tc.tile_pool(name="ncp", bufs=1, space="PSUM") as ncp,
            tc.tile_pool(name="outp", bufs=1, space="PSUM") as outp,
        ):
            ident = cst.tile([128, 128], bf16)
            make_identity(nc, ident[:])
            ones_t = cst.tile([1, 128], bf16)
            nc.gpsimd.memset(ones_t[:], 1.0)
            magic_t = cst.tile([NB, 4, 8], i32)
            nc.gpsimd._memset_packed(magic_t[:], MAGIC)

            # ---- persistent input tiles; all DMAs issued up front ----
            pwh = cst.tile([128, NGRP, 8, 192], f8)
            pwl = cst.tile([65, NGRP, 8, 192], f8)
            ncw = cst.tile([128, NKT, 640], f8)
            wout_sb = cst.tile([128, NFT + NFT_NC, OUT], bf16)
            bout_sb = cst.tile([1, OUT], bf16)

            # Issue order == rough arrival order == consumption order.
            # Fine ~0.5-1MB chunks measurably raise DMA throughput
            # (~295GB/s vs ~225 at coarse chunks); interleave nc stream.
            pw_chunks = [(a, min(a + 2, NGRP)) for a in range(0, NGRP, 2)]
            kk = (NKT + 7) // 8
            nc_chunks = [(a, min(a + kk, NKT)) for a in range(0, NKT, kk)]
            order = ["p", "p", "w", "p", "p", "n", "p", "n", "p", "n", "p",
                     "n", "p", "n", "p", "n", "p", "n", "p", "n", "p", "p"]
            pi = ni = 0
            for tok in order:
                if tok == "p" and pi < len(pw_chunks):
                    a, b = pw_chunks[pi]; pi += 1
                    nc.sync.dma_start(pwh[:, a:b], pwh_d[:, a:b])
                    nc.sync.dma_start(pwl[:, a:b], pwl_d[:, a:b])
                elif tok == "w":
                    nc.sync.dma_start(wout_sb[:], wout_d[:])
                    nc.sync.dma_start(bout_sb[:], bout_d[:])
                elif tok == "n" and ni < len(nc_chunks):
                    a, b = nc_chunks[ni]; ni += 1
                    nc.sync.dma_start(ncw[:, a:b], ncw_d[:, a:b])
            while pi < len(pw_chunks):
                a, b = pw_chunks[pi]; pi += 1
                nc.sync.dma_start(pwh[:, a:b], pwh_d[:, a:b])
                nc.sync.dma_start(pwl[:, a:b], pwl_d[:, a:b])
            while ni < len(nc_chunks):
                a, b = nc_chunks[ni]; ni += 1
                nc.sync.dma_start(ncw[:, a:b], ncw_d[:, a:b])

            nc_ps = ncp.tile([NB, HID], f32)
            out_ps = outp.tile([NB, OUT], f32)
            ss_all = cst.tile([NB, NGRP, 8], f32)
            inv_all = cst.tile([NB, NGRP, 8], f32)

            def rsqrt_batch(ga, gb):
                # inv = rsqrt(ss) on DVE, batched over groups [ga:gb):
                # y0 = bits(MAGIC - (bits(ss)>>1)); 1 Newton step:
                # inv = y0*(1.5 - (ss/2)*y0^2). Max rel err ~0.17%.
                w = gb - ga
                ss = ss_all[:, ga:gb, :]
                shv = ivp.tile([NB, w, 8], i32)
                nc.vector.tensor_scalar(shv[:], ss.bitcast(i32), 1, None,
                                        ALU.arith_shift_right)
                y0i = ivp.tile([NB, w, 8], i32)
                nc.vector.tensor_tensor(y0i[:], magic_t[:, 0:w, :], shv[:],
                                        ALU.subtract)
                y0 = y0i[:].bitcast(f32)
                x2 = ivp.tile([NB, w, 8], f32)
                nc.vector.tensor_scalar(x2[:], ss, 0.5, None, ALU.mult)
                u = ivp.tile([NB, w, 8], f32)
                nc.vector.tensor_tensor(u[:], x2[:], y0, ALU.mult)
                nc.vector.tensor_tensor(u[:], u[:], y0, ALU.mult)
                nc.vector.tensor_scalar(u[:], u[:], -1.0, 1.5, ALU.mult, ALU.add)
                nc.vector.tensor_tensor(inv_all[:, ga:gb, :], u[:], y0, ALU.mult)

            # nc-branch DoubleRow matmul emitter: kt-pairs x 2 hid halves
            nc_sched = [(i, h) for i in range(NKP) for h in range(2)]
            nc_emitted = 0

            def emit_nc(upto):
                nonlocal nc_emitted
                while nc_emitted < min(upto, len(nc_sched)):
                    i, h = nc_sched[nc_emitted]
                    nc.tensor.matmul(
                        nc_ps[:, 256 * h:256 * h + 256],
                        ncw[:, 2 * i:2 * i + 2, 0:128],
                        ncw[:, 2 * i:2 * i + 2, 128 + 256 * h:384 + 256 * h],
                        start=(i == 0), stop=(i == NKP - 1),
                        perf_mode=DR,
                    )
                    nc_emitted += 1

            # ---------- fused main loop: group-granular round-robin with a
            # 4-step software-pipeline: step i runs S1(i) | S3(i-4) | S4(i-5)
            # so every op consumes inputs produced >= 3 steps earlier — the
            # matmul->square->reduce->rsqrt->mul->sigmoid->transpose chain
            # latency is fully hidden and DVE/ACT stay back-to-back busy.
            DEPTH = 4
            t_tiles = {}
            s8_tiles = {}

            def S1(g):
                t_ps = tp.tile([NB, 8, K], f32)
                t_tiles[g] = t_ps
                for j in range(8):
                    nc.tensor.matmul(
                        t_ps[:, j, :], pwh[:, g, j, 0:128],
                        pwh[:, g, j, 128:192], start=True, stop=False)
                    nc.tensor.matmul(
                        t_ps[:, j, :], pwl[:, g, j, 0:128],
                        pwl[:, g, j, 128:192], start=False, stop=True)
                sq = sqp.tile([NB, 8, K], bf16)
                nc.scalar.activation(sq[:], t_ps[:], AF.Square)
                nc.vector.tensor_reduce(ss_all[:, g, :], sq[:],
                                        axis=mybir.AxisListType.X, op=ALU.add)

            def S3(g):
                t_ps = t_tiles.pop(g)
                tn = tnp.tile([NB, 8, K], bf16)
                nc.vector.tensor_mul(
                    tn[:], t_ps[:],
                    inv_all[:, g, :].broadcast_to((NB, 8, K)))
                s8 = s8p.tile([NB, 8, K], bf16)
                nc.scalar.activation(s8[:], tn[:], AF.Sigmoid)
                s8_tiles[g] = s8

            def S4(g):
                s8 = s8_tiles.pop(g)
                st_ps = stp.tile([128, 4, NB], f32)
                for jj in range(4):
                    nc.tensor.matmul(st_ps[:, jj, :],
                                     s8[:, 2 * jj:2 * jj + 2, :],
                                     ident[:], start=True, stop=True)
                hT = htp.tile([128, 4, NB], bf16)
                if g % 2 == 0:
                    nc.vector.tensor_copy(hT[:], st_ps[:])
                else:
                    nc.scalar.copy(hT[:], st_ps[:])
                for jj in range(4):
                    nc.tensor.matmul(out_ps[:], hT[:, jj, :],
                                     wout_sb[:, 4 * g + jj, :],
                                     start=(g == 0 and jj == 0), stop=False)

            L = len(nc_sched)
            nc_after = {17: L // 6, 19: L // 3, 21: L // 2, 23: (2 * L) // 3,
                        25: (5 * L) // 6}
            for i in range(NGRP + DEPTH + 1):
                if i < NGRP:
                    S1(i)
                    if i % 4 == 3 or i == NGRP - 1:
                        rsqrt_batch((i // 4) * 4, i + 1)
                if 0 <= i - DEPTH < NGRP:
                    S3(i - DEPTH)
                if 0 <= i - DEPTH - 1 < NGRP:
                    S4(i - DEPTH - 1)
                emit_nc(nc_after.get(i, 0))
            emit_nc(len(nc_sched))

            # ---------- tail: nc sigmoid (undo x64 weight scale), out -------
            s_nc = cst.tile([NB, HID], bf16)
            nc.scalar.activation(s_nc[:], nc_ps[:], AF.Sigmoid, scale=1.0 / 64.0)
            st_ps = stp.tile([128, 4, NB], f32)
            for i in range(NFT_NC):
                nc.tensor.matmul(st_ps[:, i, :], s_nc[:, i * 128:(i + 1) * 128],
                                 ident[:], start=True, stop=True)
            hT = htp.tile([128, 4, NB], bf16)
            nc.vector.tensor_copy(hT[:, 0:NFT_NC, :], st_ps[:, 0:NFT_NC, :])
            for i in range(NFT_NC):
                nc.tensor.matmul(out_ps[:], hT[:, i, :], wout_sb[:, NFT + i, :],
                                 start=False, stop=False)
            nc.tensor.matmul(out_ps[:], ones_t[:], bout_sb[:],
                             start=False, stop=True)

            out_sb = cst.tile([NB, OUT], f32)
            nc.vector.tensor_copy(out_sb[:], out_ps[:])
            nc.sync.dma_start(out_d[:], out_sb[:])

    nc.compile()
    return nc


def _prep(inputs):
    x = np.asarray(inputs["x"], np.float32)
    pathway_ids = np.asarray(inputs["pathway_ids"]).astype(np.int64)
    nc_ids = np.asarray(inputs["nc_ids"]).astype(np.int64)
    W_path = np.asarray(inputs["W_path"], np.float32)
    b_path = np.asarray(inputs["b_path"], np.float32)
    W_nc = np.asarray(inputs["W_nc"], np.float32)
    b_nc = np.asarray(inputs["b_nc"], np.float32)
    W_out = np.asarray(inputs["W_out"], np.float32)
    b_out = np.asarray(inputs["b_out"], np.float32)

    n = x.shape[0]
    xt = np.ascontiguousarray(x.reshape(n, G * C).T)            # [60000, n]
    xf = xt.astype(F8)

    # pathway gather: contraction row i of pathway p = gene pathway_ids[p, i//3], channel i%3
    pidx = ((pathway_ids * 3)[:, :, None] + np.arange(3)).reshape(P, KC)
    prows = xf[pidx.reshape(-1)].reshape(P, KC, n)              # [200, 192, n]
    ph = prows[:, 0:128, :]                                     # [200, 128, n]
    pl = np.concatenate([prows[:, 128:KC, :],
                         np.ones((P, 1, n), F8)], axis=1)       # [200, 65, n]

    # weights x8 (cancelled exactly by the per-pathway L2 normalize)
    w8 = (8.0 * W_path).astype(np.float32)                      # [200, 192, 64]
    wphi = np.ascontiguousarray(w8[:, 0:128, :].transpose(1, 0, 2)).astype(F8)
    wplo = np.ascontiguousarray(
        np.concatenate([w8[:, 128:KC, :], (8.0 * b_path)[:, None, :]], axis=1)
        .transpose(1, 0, 2)).astype(F8)                         # [65, 200, 64]

    # nc gather rows: dedup repeated gene ids (fold their W_nc rows by
    # summation — exact, just a reordering of the contraction), append the
    # ones row for the bias, zero-pad to a whole number of k-tile pairs.
    uniq, inverse = np.unique(nc_ids, return_inverse=True)
    U = len(uniq)
    nrows = U * C + 1
    NKT = 2 * ((nrows + 255) // 256)
    NKROWS = NKT * 128
    Wf = np.zeros((U, C, HID), np.float32)
    np.add.at(Wf, inverse, W_nc.reshape(NCG, C, HID))

    nidx = ((uniq * 3)[:, None] + np.arange(3)).reshape(-1)
    ncr = np.zeros((NKROWS, n), F8)
    ncr[:U * C] = xf[nidx]
    ncr[U * C] = 1.0

    wnca = np.zeros((NKROWS, HID), np.float32)
    wnca[:U * C] = 64.0 * Wf.reshape(U * C, HID)
    wnca[U * C] = 64.0 * b_nc
    wnc8 = np.ascontiguousarray(
        wnca.reshape(NKT, 128, HID).transpose(1, 0, 2)).astype(F8)  # [128, NKT, 512]

    wout_t = np.ascontiguousarray(
        W_out.reshape(NFT + NFT_NC, 128, OUT).transpose(1, 0, 2)).astype(BF)
    bout = b_out.reshape(1, OUT).astype(BF)

    wphi_g = wphi.reshape(128, NGRP, 8, K)
    wplo_g = wplo.reshape(65, NGRP, 8, K)
    in_maps = []
    for c in range(NCORES):
        sl = slice(c * NB, (c + 1) * NB)
        pdhi = np.ascontiguousarray(
            ph[:, :, sl].transpose(1, 0, 2)).reshape(128, NGRP, 8, 128)
        pdlo = np.ascontiguousarray(
            pl[:, :, sl].transpose(1, 0, 2)).reshape(65, NGRP, 8, 128)
        pwh = np.concatenate([pdhi, wphi_g], axis=3)            # [128,25,8,192]
        pwl = np.concatenate([pdlo, wplo_g], axis=3)            # [65,25,8,192]
        ncd = np.ascontiguousarray(
            ncr[:, sl].reshape(NKT, 128, NB).transpose(1, 0, 2))  # [128,NKT,128]
        ncw = np.concatenate([ncd, wnc8], axis=2)               # [128,NKT,640]
        in_maps.append({
            "pwh": np.ascontiguousarray(pwh),
            "pwl": np.ascontiguousarray(pwl),
            "ncw": np.ascontiguousarray(ncw),
            "wout": wout_t,
            "bout": bout,
        })
    return in_maps, NKT


def kernel(**inputs):
    in_maps, NKT = _prep(inputs)
    key = ("nc", NKT)
    if key not in _CACHE:
        _CACHE[key] = _build(NKT)
    nc = _CACHE[key]
    res = run_bass_kernel_spmd(nc, in_maps, list(range(NCORES)), **_CACHE.get("run_kwargs", {}))
    _CACHE["last_result"] = res
    return np.concatenate([res.results[c]["out"] for c in range(NCORES)], axis=0)


if __name__ == "__main__":
    print("building only...")
    _build(104)
    print("build OK")



# revision 14
# speedup vs baseline: 1.1381x; 1.1381x over previous
"""Trainium2 Bass kernel for DeepMultiOmicPathwayNet.

Model (per batch row n):
  g    = x[n, pathway_ids, :]                  -> [P=200, K*C=192]
  t    = einsum('pi,pio->po', g, W_path) + b_path      (per-pathway linear)
  t    = t / ||t||_2 (row L2 over each pathway's 64 outputs)
  ncb  = x[n, nc_ids, :].flatten() @ W_nc + b_nc       ([15000] @ [15000,512])
  h    = sigmoid(concat(t.flatten(), ncb))             ([13312])
  out  = h @ W_out + b_out                             ([20])

Strategy: data-parallel over batch N=1024 across 8 cores (128 rows/core).
DMA-bound problem; v6 design:
  - All large tensors ship as fp8e4m3 (~2x fewer HBM bytes than bf16).
    W_path/b_path pre-scaled x8 (exactly cancelled by the L2 norm);
    W_nc/b_nc pre-scaled x64 (compensated via sigmoid's scale=1/64).
  - nc gather dedup: repeated gene ids are shipped once, their W_nc rows
    folded by summation on the host (exact).
  - nc-branch matmuls use fp8 DoubleRow perf mode (0.5 cycles/row).
  - Data and weights are packed into shared DRAM tensors so each chunk is
    ONE large DMA (~23 DMAs total); pathway data first in the stream (it
    feeds the long DVE/ACT elementwise pipeline), nc stream last.
  - Elementwise work batched over PAIRS of 8-pathway groups ([128,1024]
    per op) to halve instruction count and amortize access latencies.
  - rsqrt via Quake bit-trick + 1 Newton step on DVE (max err ~0.17%),
    batched per 4-group sub-phase: no ACT table swaps, no phase split.
  - Software pipelining: the transpose/copy/W_out stage of sub-phase k-1
    is emitted between sub-phase k's reduce and normalize stages.
"""
import numpy as np
import ml_dtypes

import concourse.bass as bass
import concourse.bacc as bacc
import concourse.tile as tile
import concourse.mybir as mybir
from concourse.bass_utils import run_bass_kernel_spmd
from concourse.masks import make_identity

bf16 = mybir.dt.bfloat16
f32 = mybir.dt.float32
i32 = mybir.dt.int32
f8 = mybir.dt.float8e4
BF = ml_dtypes.bfloat16
F8 = ml_dtypes.float8_e4m3
AF = mybir.ActivationFunctionType
ALU = mybir.AluOpType
DR = mybir.MatmulPerfMode.DoubleRow

N, G, C = 1024, 20000, 3
P, K = 200, 64
KC = K * C              # 192
NCG = 5000              # non-cancer genes
HID = 512
OUT = 20
NB = 128                # batch rows per core
NCORES = 8
NGRP = P // 8           # 25 groups of 8 pathways
NFT = P * K // 128      # 100 feature tiles from pathways
NFT_NC = HID // 128     # 4 feature tiles from nc branch
MAGIC = 0x5F3759DF      # fast inverse sqrt

_CACHE = {}


def _build(NKT):
    NKP = NKT // 2      # DoubleRow k-tile pairs
    nc = bacc.Bacc(None, target_bir_lowering=False)

    # data + weights packed per pathway: [..., 0:128]=x rows, [128:192]=W
    pwh_d = nc.declare_dram_parameter("pwh", [128, NGRP, 8, 192], f8, isOutput=False)
    pwl_d = nc.declare_dram_parameter("pwl", [65, NGRP, 8, 192], f8, isOutput=False)
    # nc branch packed per k-tile: [..., 0:128]=x rows, [128:640]=W_nc
    ncw_d = nc.declare_dram_parameter("ncw", [128, NKT, 640], f8, isOutput=False)
    wout_d = nc.declare_dram_parameter("wout", [128, NFT + NFT_NC, OUT], f8, isOutput=False)
    bout_d = nc.declare_dram_parameter("bout", [1, OUT], bf16, isOutput=False)
    out_d = nc.declare_dram_parameter("out", [NB, OUT], f32, isOutput=True)

    with tile.TileContext(nc) as tc:
        with (
            tc.tile_pool(name="cst", bufs=1) as cst,
            tc.tile_pool(name="sqp", bufs=2) as sqp,
            tc.tile_pool(name="tnp", bufs=2) as tnp,
            tc.tile_pool(name="s8p", bufs=4) as s8p,
            tc.tile_pool(name="htp", bufs=2) as htp,
            tc.tile_pool(name="ivp", bufs=2) as ivp,
            tc.tile_pool(name="tp", bufs=5, space="PSUM") as tp,
            tc.tile_pool(name="stp", bufs=1, space="PSUM") as stp,
            tc.tile_pool(name="ncp", bufs=1, space="PSUM") as ncp,
            tc.tile_pool(name="outp", bufs=1, space="PSUM") as outp,
        ):
            ident = cst.tile([128, 128], bf16)
            make_identity(nc, ident[:])
            ones_t = cst.tile([1, 128], bf16)
            nc.gpsimd.memset(ones_t[:], 1.0)
            magic_t = cst.tile([NB, 4, 8], i32)
            nc.gpsimd._memset_packed(magic_t[:], MAGIC)

            # ---- persistent input tiles; all DMAs issued up front ----
            pwh = cst.tile([128, NGRP, 8, 192], f8)
            pwl = cst.tile([65, NGRP, 8, 192], f8)
            ncw = cst.tile([128, NKT, 640], f8)
            wout_sb = cst.tile([128, NFT + NFT_NC, OUT], f8)
            bout_sb = cst.tile([1, OUT], bf16)

            # Issue order == rough arrival order == consumption order.
            gsplit = [0, 2, 7, 13, 19, NGRP]
            ksplit = [2 * round(NKT * i / 10) for i in range(6)]
            for ci in range(5):
                a, b = gsplit[ci], gsplit[ci + 1]
                nc.sync.dma_start(pwh[:, a:b], pwh_d[:, a:b])
                nc.sync.dma_start(pwl[:, a:b], pwl_d[:, a:b])
                if ci == 0:
                    nc.sync.dma_start(wout_sb[:], wout_d[:])
                    nc.sync.dma_start(bout_sb[:], bout_d[:])
            for ci in range(5):
                ka, kb = ksplit[ci], ksplit[ci + 1]
                km = (ka + kb) // 2
                nc.sync.dma_start(ncw[:, ka:km], ncw_d[:, ka:km])
                nc.sync.dma_start(ncw[:, km:kb], ncw_d[:, km:kb])

            nc_ps = ncp.tile([NB, HID], f32)
            out_ps = outp.tile([NB, OUT], f32)
            ss_all = cst.tile([NB, NGRP, 8], f32)
            inv_all = cst.tile([NB, NGRP, 8], f32)

            def rsqrt_batch(ga, gb):
                # inv = rsqrt(ss) on DVE, batched over groups [ga:gb):
                # y0 = bits(MAGIC - (bits(ss)>>1)); 1 Newton step:
                # inv = y0*(1.5 - (ss/2)*y0^2). Max rel err ~0.17%.
                w = gb - ga
                ss = ss_all[:, ga:gb, :]
                shv = ivp.tile([NB, w, 8], i32)
                nc.vector.tensor_scalar(shv[:], ss.bitcast(i32), 1, None,
                                        ALU.arith_shift_right)
                y0i = ivp.tile([NB, w, 8], i32)
                nc.vector.tensor_tensor(y0i[:], magic_t[:, 0:w, :], shv[:],
                                        ALU.subtract)
                y0 = y0i[:].bitcast(f32)
                x2 = ivp.tile([NB, w, 8], f32)
                nc.vector.tensor_scalar(x2[:], ss, 0.5, None, ALU.mult)
                u = ivp.tile([NB, w, 8], f32)
                nc.vector.tensor_tensor(u[:], x2[:], y0, ALU.mult)
                nc.vector.tensor_tensor(u[:], u[:], y0, ALU.mult)
                nc.vector.tensor_scalar(u[:], u[:], -1.0, 1.5, ALU.mult, ALU.add)
                nc.vector.tensor_tensor(inv_all[:, ga:gb, :], u[:], y0, ALU.mult)

            # nc-branch DoubleRow matmul emitter: kt-pairs x 2 hid halves
            nc_sched = [(i, h) for i in range(NKP) for h in range(2)]
            nc_emitted = 0

            def emit_nc(upto):
                nonlocal nc_emitted
                while nc_emitted < min(upto, len(nc_sched)):
                    i, h = nc_sched[nc_emitted]
                    nc.tensor.matmul(
                        nc_ps[:, 256 * h:256 * h + 256],
                        ncw[:, 2 * i:2 * i + 2, 0:128],
                        ncw[:, 2 * i:2 * i + 2, 128 + 256 * h:384 + 256 * h],
                        start=(i == 0), stop=(i == NKP - 1),
                        perf_mode=DR,
                    )
                    nc_emitted += 1

            # ---------- fused main loop: group-granular round-robin with a
            # 4-step software-pipeline: step i runs S1(i) | S3(i-4) | S4(i-5)
            # so every op consumes inputs produced >= 3 steps earlier — the
            # matmul->square->reduce->rsqrt->mul->sigmoid->transpose chain
            # latency is fully hidden and DVE/ACT stay back-to-back busy.
            DEPTH = 4
            t_tiles = {}
            s8_tiles = {}

            def S1(g):
                t_ps = tp.tile([NB, 8, K], f32)
                t_tiles[g] = t_ps
                for j in range(8):
                    nc.tensor.matmul(
                        t_ps[:, j, :], pwh[:, g, j, 0:128],
                        pwh[:, g, j, 128:192], start=True, stop=False)
                    nc.tensor.matmul(
                        t_ps[:, j, :], pwl[:, g, j, 0:128],
                        pwl[:, g, j, 128:192], start=False, stop=True)
                sq = sqp.tile([NB, 8, K], bf16)
                nc.scalar.activation(sq[:], t_ps[:], AF.Square)
                nc.vector.tensor_reduce(ss_all[:, g, :], sq[:],
                                        axis=mybir.AxisListType.X, op=ALU.add)

            def S3(g):
                t_ps = t_tiles.pop(g)
                tn = tnp.tile([NB, 8, K], bf16)
                nc.vector.tensor_mul(
                    tn[:], t_ps[:],
                    inv_all[:, g, :].broadcast_to((NB, 8, K)))
                s8 = s8p.tile([NB, 8, K], f8)
                nc.scalar.activation(s8[:], tn[:], AF.Tanh, scale=0.5)
                s8_tiles[g] = s8

            def S4(g):
                s8 = s8_tiles.pop(g)
                st_ps = stp.tile([128, 4, NB], f32)
                for jj in range(4):
                    nc.tensor.matmul(st_ps[:, jj, :],
                                     s8[:, 2 * jj:2 * jj + 2, :],
                                     ident[:], start=True, stop=True)
                hT = htp.tile([128, 4, NB], f8)
                if g % 2 == 0:
                    nc.vector.tensor_copy(hT[:], st_ps[:])
                else:
                    nc.scalar.copy(hT[:], st_ps[:])
                for jj in range(2):
                    nc.tensor.matmul(out_ps[:], hT[:, 2 * jj:2 * jj + 2, :],
                                     wout_sb[:, 4 * g + 2 * jj:4 * g + 2 * jj + 2, :],
                                     start=(g == 0 and jj == 0), stop=False,
                                     perf_mode=DR)

            L = len(nc_sched)
            nc_after = {17: L // 6, 19: L // 3, 21: L // 2, 23: (2 * L) // 3,
                        25: (5 * L) // 6}
            for i in range(NGRP + DEPTH + 1):
                if i < NGRP:
                    S1(i)
                    if i % 4 == 3 or i == NGRP - 1:
                        rsqrt_batch((i // 4) * 4, i + 1)
                if 0 <= i - DEPTH < NGRP:
                    S3(i - DEPTH)
                if 0 <= i - DEPTH - 1 < NGRP:
                    S4(i - DEPTH - 1)
                emit_nc(nc_after.get(i, 0))
            emit_nc(len(nc_sched))

            # ---------- tail: nc tanh(pre/128) (undo x64 weight scale), out --
            s_nc = cst.tile([NB, HID], f8)
            nc.scalar.activation(s_nc[:], nc_ps[:], AF.Tanh, scale=1.0 / 128.0)
            st_ps = stp.tile([128, 4, NB], f32)
            for i in range(NFT_NC):
                nc.tensor.matmul(st_ps[:, i, :], s_nc[:, i * 128:(i + 1) * 128],
                                 ident[:], start=True, stop=True)
            hT = htp.tile([128, 4, NB], f8)
            nc.vector.tensor_copy(hT[:, 0:NFT_NC, :], st_ps[:, 0:NFT_NC, :])
            for i in range(2):
                nc.tensor.matmul(out_ps[:], hT[:, 2 * i:2 * i + 2, :],
                                 wout_sb[:, NFT + 2 * i:NFT + 2 * i + 2, :],
                                 start=False, stop=False, perf_mode=DR)
            nc.tensor.matmul(out_ps[:], ones_t[:], bout_sb[:],
                             start=False, stop=True)

            out_sb = cst.tile([NB, OUT], f32)
            nc.vector.tensor_scalar(out_sb[:], out_ps[:], 1.0 / 64.0, None,
                                    ALU.mult)
            nc.sync.dma_start(out_d[:], out_sb[:])

    nc.compile()
    return nc


def _prep(inputs):
    x = np.asarray(inputs["x"], np.float32)
    pathway_ids = np.asarray(inputs["pathway_ids"]).astype(np.int64)
    nc_ids = np.asarray(inputs["nc_ids"]).astype(np.int64)
    W_path = np.asarray(inputs["W_path"], np.float32)
    b_path = np.asarray(inputs["b_path"], np.float32)
    W_nc = np.asarray(inputs["W_nc"], np.float32)
    b_nc = np.asarray(inputs["b_nc"], np.float32)
    W_out = np.asarray(inputs["W_out"], np.float32)
    b_out = np.asarray(inputs["b_out"], np.float32)

    n = x.shape[0]
    xt = np.ascontiguousarray(x.reshape(n, G * C).T)            # [60000, n]
    xf = xt.astype(F8)

    # pathway gather: contraction row i of pathway p = gene pathway_ids[p, i//3], channel i%3
    pidx = ((pathway_ids * 3)[:, :, None] + np.arange(3)).reshape(P, KC)
    prows = xf[pidx.reshape(-1)].reshape(P, KC, n)              # [200, 192, n]
    ph = prows[:, 0:128, :]                                     # [200, 128, n]
    pl = np.concatenate([prows[:, 128:KC, :],
                         np.ones((P, 1, n), F8)], axis=1)       # [200, 65, n]

    # weights x8 (cancelled exactly by the per-pathway L2 normalize)
    w8 = (8.0 * W_path).astype(np.float32)                      # [200, 192, 64]
    wphi = np.ascontiguousarray(w8[:, 0:128, :].transpose(1, 0, 2)).astype(F8)
    wplo = np.ascontiguousarray(
        np.concatenate([w8[:, 128:KC, :], (8.0 * b_path)[:, None, :]], axis=1)
        .transpose(1, 0, 2)).astype(F8)                         # [65, 200, 64]

    # nc gather rows: dedup repeated gene ids (fold their W_nc rows by
    # summation — exact, just a reordering of the contraction), append the
    # ones row for the bias, zero-pad to a whole number of k-tile pairs.
    uniq, inverse = np.unique(nc_ids, return_inverse=True)
    U = len(uniq)
    nrows = U * C + 1
    NKT = 2 * ((nrows + 255) // 256)
    NKROWS = NKT * 128
    Wf = np.zeros((U, C, HID), np.float32)
    np.add.at(Wf, inverse, W_nc.reshape(NCG, C, HID))

    nidx = ((uniq * 3)[:, None] + np.arange(3)).reshape(-1)
    ncr = np.zeros((NKROWS, n), F8)
    ncr[:U * C] = xf[nidx]
    ncr[U * C] = 1.0

    wnca = np.zeros((NKROWS, HID), np.float32)
    wnca[:U * C] = 64.0 * Wf.reshape(U * C, HID)
    wnca[U * C] = 64.0 * b_nc
    wnc8 = np.ascontiguousarray(
        wnca.reshape(NKT, 128, HID).transpose(1, 0, 2)).astype(F8)  # [128, NKT, 512]

    wout_t = np.ascontiguousarray(
        (32.0 * W_out).reshape(NFT + NFT_NC, 128, OUT).transpose(1, 0, 2)).astype(F8)
    bb = 64.0 * (b_out + 0.5 * W_out.sum(axis=0))
    bout = bb.reshape(1, OUT).astype(BF)

    wphi_g = wphi.reshape(128, NGRP, 8, K)
    wplo_g = wplo.reshape(65, NGRP, 8, K)
    in_maps = []
    for c in range(NCORES):
        sl = slice(c * NB, (c + 1) * NB)
        pdhi = np.ascontiguousarray(
            ph[:, :, sl].transpose(1, 0, 2)).reshape(128, NGRP, 8, 128)
        pdlo = np.ascontiguousarray(
            pl[:, :, sl].transpose(1, 0, 2)).reshape(65, NGRP, 8, 128)
        pwh = np.concatenate([pdhi, wphi_g], axis=3)            # [128,25,8,192]
        pwl = np.concatenate([pdlo, wplo_g], axis=3)            # [65,25,8,192]
        ncd = np.ascontiguousarray(
            ncr[:, sl].reshape(NKT, 128, NB).transpose(1, 0, 2))  # [128,NKT,128]
        ncw = np.concatenate([ncd, wnc8], axis=2)               # [128,NKT,640]
        in_maps.append({
            "pwh": np.ascontiguousarray(pwh),
            "pwl": np.ascontiguousarray(pwl),
            "ncw": np.ascontiguousarray(ncw),
            "wout": wout_t,
            "bout": bout,
        })
    return in_maps, NKT


def kernel(**inputs):
    in_maps, NKT = _prep(inputs)
    key = ("nc", NKT)
    if key not in _CACHE:
        _CACHE[key] = _build(NKT)
    nc = _CACHE[key]
    res = run_bass_kernel_spmd(nc, in_maps, list(range(NCORES)), **_CACHE.get("run_kwargs", {}))
    _CACHE["last_result"] = res
    return np.concatenate([res.results[c]["out"] for c in range(NCORES)], axis=0)


if __name__ == "__main__":
    print("building only...")
    _build(104)
    print("build OK")

